# revision 3
# baseline (speedup 1.0000x reference)
"""Trainium2 Bass kernel v2 for DigitConvolutionalModel.

Model: x[B,784] -> 3x3 valid conv -> [B,676] -> Linear(676,100)+relu ->
Linear(100,10)+relu -> Linear(10,10).  Conv folds into W1 (W1f = C@w1),
so the model is a 3-layer MLP 784 -> 100 -> 10 -> 10.

Key ideas vs v1:
- x main features (0..767) shipped as fp8 e3m4 (1 byte): halves HBM
  traffic; PE consumes e3m4 moving operand directly against bf16
  stationary weights (mixed-dtype matmul, 1 row/cycle).  Quant noise
  ~1.4e-2 on the max-metric (gate 2e-2), measured host-side.
- The K=16 feature tail + Linear2 + Linear3 all fold into ONE combined
  126-lane matmul per supertile.  Output-column permutation puts the h2
  block INSIDE the h1 range so all relus are one quadrant-aligned ACT:
    psum/stationary cols: [h1lo 0:96 | h2(t-1) 96:106 | h1hi 106:110 | y(t-2) 110:120]
    moving ct lanes:      [h1r_lo 0:96 | h2r 96:106 | h1r_hi 106:110 | xtail 110:126]
  Post-combined work per supertile: ONE ACT relu over [0:110] (per-
  partition bias = [b1lo|b2|b1hi]) writing the next supertile's moving
  lanes, one DVE add for y (+b3, read expanded to base 96), one store.
  The h1 main chunks accumulate into PSUM cols 0:110 (zeros in 96:106);
  chunk 0's stationary is 120 wide so start=True zeroes the h2/y cols.
  PE work drops from 9 to 7 512-row passes per supertile (the floor:
  768/128 + 126/128 lanes).
"""

import numpy as np
import ml_dtypes

import concourse.bacc as bacc
import concourse.tile as tile
from concourse.tile import add_dep_helper
from concourse import mybir
from concourse.bass_utils import run_bass_kernel_spmd

N_CORES = 8
B = 65536
BC = B // N_CORES  # 8192 rows per core
TN = 512           # batch columns per supertile
NT = BC // TN      # 16 supertiles per core
NKC = 6            # full 128-feature chunks (0..767)
KT = 16            # tail features (768..783)
NF = 784
H1 = 100
HO = 10
F32 = mybir.dt.float32
BF16 = mybir.dt.bfloat16
E3 = mybir.dt.float8e3
NP_BF16 = ml_dtypes.bfloat16
NP_E3 = ml_dtypes.float8_e3m4

WARMUP = 16

# weight blob columns (bf16)
C_CH0 = 0                # [128, 120] chunk0 (cols 96:106 + 110:120 zero)
C_CHK = 120              # chunks 1..5, [128,110] each
C_CS = 670               # combined stationary [126, 120]
C_BIAS = 790             # f32 byte pairs: rows [b1lo|b2|b1hi|b3]
WBW = 792


def _build_nc():
    nc = bacc.Bacc(None, target_bir_lowering=False)

    xm_d = nc.dram_tensor("xm", [NT, 128, NKC, TN], E3, kind="ExternalInput")
    xt_d = nc.dram_tensor("xt", [NT, KT, TN], BF16, kind="ExternalInput")
    wb_d = nc.dram_tensor("wblob", [128, WBW], BF16, kind="ExternalInput")
    yt_d = nc.dram_tensor("yt", [HO, BC], F32, kind="ExternalOutput")

    relu = mybir.ActivationFunctionType.Relu

    with tile.TileContext(nc) as tc:
        with (
            tc.tile_pool(name="const", bufs=1) as cpool,
            tc.tile_pool(name="xm", bufs=5) as xmpool,
            tc.tile_pool(name="cmb", bufs=4) as cmbpool,
            tc.tile_pool(name="ot", bufs=3) as otpool,
            tc.tile_pool(name="ps1", bufs=4, space="PSUM") as ps1,
        ):
            wb_s = cpool.tile([128, WBW], BF16, tag="wb")

            # bias rows follow the permuted output-column layout; the DVE
            # y-add reads [96:120] (quadrant base), its junk lanes get the
            # b2/b1hi rows which is harmless scratch.
            bias_ap = wb_s[0:110, C_BIAS:C_BIAS + 2].bitcast(F32)
            b3e_ap = wb_s[96:120, C_BIAS:C_BIAS + 2].bitcast(F32)
            cs = wb_s[0:126, C_CS:C_CS + 120]

            prev_mm = [None]

            def mm(out_ap, lhsT_ap, rhs_ap, start, stop, ldw=True):
                m = nc.tensor.matmul(out_ap, lhsT_ap, rhs_ap,
                                     start=start, stop=stop)
                if not ldw:
                    m.ins.ldweights = False
                if prev_mm[0] is not None:
                    add_dep_helper(m.ins, prev_mm[0], sync=False,
                                   reason="pe program order")
                prev_mm[0] = m.ins
                return m

            # Warmup: dummy matmuls ramp the PE p-state during NEFF
            # startup (instruction loads + first DMAs, ~9us).
            wsc = cpool.tile([128, TN], BF16, tag="wsc")
            wp0 = ps1.tile([120, TN], F32, tag="p1")
            wp1 = ps1.tile([120, TN], F32, tag="p1")
            wfirst = nc.tensor.matmul(wp0[:], wsc[:, 0:120], wsc[:],
                                      start=True, stop=True)
            prev_mm[0] = wfirst.ins
            for i in range(1, WARMUP):
                w_mm = nc.tensor.matmul((wp0 if i % 2 else wp1)[:],
                                        wsc[:, 0:120], wsc[:],
                                        start=True, stop=True)
                w_mm.ins.ldweights = False
                add_dep_helper(w_mm.ins, prev_mm[0], sync=False,
                               reason="warmup order")
                prev_mm[0] = w_mm.ins
            # WAR write: warmup multiplies garbage on purpose; memset only
            # exists to satisfy tile allocation and runs after the reads.
            nc.vector.memset(wsc[:], 0.0)

            # prefetch.  Sync queue starts earliest: first half of xm(0)
            # goes out first so mains(0) can start ASAP, then the weight
            # blob (needed by the same matmul), then the rest.  Tails ride
            # the scalar queue (tiny; ACT engine is mostly idle).
            xm_s: dict[int, object] = {}
            ct: dict[int, object] = {}
            xm_s[0] = xmpool.tile([128, NKC, TN], E3, tag="xm", name="xm0")
            nc.sync.dma_start(xm_s[0][:, 0:3, :], xm_d[0, :, 0:3, :])
            nc.sync.dma_start(wb_s[:], wb_d[:])
            nc.sync.dma_start(xm_s[0][:, 3:6, :], xm_d[0, :, 3:6, :])
            for t in range(1, 4):
                xm_s[t] = xmpool.tile([128, NKC, TN], E3, tag="xm", name=f"xm{t}")
                nc.sync.dma_start(xm_s[t][:], xm_d[t])
            for t in range(3):
                ct[t] = cmbpool.tile([126, TN], BF16, tag="ct", name=f"ct{t}")
                if t == 0:
                    # h1r/h2r lanes of ct[0] are never written: zero them
                    # so combined(0) (which contracts ALL lanes for every
                    # output column) can't pull NaN garbage into h1(0).
                    nc.vector.memset(ct[0][0:110, :], 0.0)
                nc.scalar.dma_start(ct[t][110:126, :], xt_d[t])

            p1: dict[int, object] = {}
            ot_cur = [None]

            def emit_post(t):
                """ACT/DVE/store work after combined(t)."""
                # ONE relu for h1lo|h2|h1hi: the permuted column layout
                # makes them contiguous in p1[0:110] with per-row bias.
                if t <= 16:
                    nc.scalar.activation(ct[t + 1][0:110, :], p1[t][0:110, :],
                                         relu, bias=bias_ap)
                # y(t-2) = p1[t][110:120] + b3, read/written as [96:120]
                # (quadrant base; lanes 96:110 are scratch)
                s = t - 2
                if s >= 0:
                    j = s % 2
                    if j == 0:
                        ot_cur[0] = otpool.tile([120, 2, TN], F32, tag="ot", name=f"ot{t}")
                    nc.vector.tensor_scalar_add(ot_cur[0][96:120, j, :],
                                                p1[t][96:120, :], b3e_ap)
                    if j == 1:
                        # last pair on sync (idle by then, lower latency)
                        eng = nc.sync if s >= 15 else nc.gpsimd
                        eng.dma_start(
                            yt_d[:, (s - 1) * TN:(s + 1) * TN],
                            ot_cur[0][110:120, :, :])

            for t in range(NT):
                # prefetch xm(t+4), allocate ct[t+3] + its tail dma
                if t + 4 < NT:
                    xm_s[t + 4] = xmpool.tile([128, NKC, TN], E3, tag="xm", name=f"xm{t+4}")
                    nc.sync.dma_start(xm_s[t + 4][:], xm_d[t + 4])
                if t + 3 <= 17:
                    ct[t + 3] = cmbpool.tile([126, TN], BF16, tag="ct", name=f"ct{t+3}")
                    if t + 3 <= 15:
                        nc.scalar.dma_start(ct[t + 3][110:126, :], xt_d[t + 3])
                    # drain tiles (16, 17) need no memsets: the sliced
                    # drain matmuls below read only their live lanes

                p1[t] = ps1.tile([120, TN], F32, tag="p1", name=f"p1_{t}")
                xm = xm_s.pop(t)
                # main chunks: chunk0 with 120-wide stationary zeroes the
                # h2/y accumulator rows
                mm(p1[t][:], wb_s[:, C_CH0:C_CH0 + 120], xm[:, 0, :],
                   start=True, stop=False)
                for k in range(1, NKC):
                    mm(p1[t][0:110, :],
                       wb_s[:, C_CHK + 110 * (k - 1):C_CHK + 110 * k],
                       xm[:, k, :], start=False, stop=False)
                # combined: tail + L2(t-1) + L3(t-2).  Full 126-lane moving
                # even at t=0/1 where the h1r/h2r lanes are garbage: those
                # only pollute output columns that are never read (y(-2),
                # y(-1), h2(-1)), keeping tile_position at (0, 0).
                mm(p1[t][:], cs, ct[t][:], start=False, stop=True)
                emit_post(t)

            # pipeline drain, sliced to live lanes only (quadrant-legal
            # bases 0 and 96): combined(16) skips the xtail lanes,
            # combined(17) reads just the h2r lanes.
            p1[16] = ps1.tile([120, TN], F32, tag="p1", name="p1_16")
            mm(p1[16][:], cs[0:110, :], ct[16][0:110, :], start=True,
               stop=True)
            emit_post(16)
            p1[17] = ps1.tile([120, TN], F32, tag="p1", name="p1_17")
            mm(p1[17][:], cs[0:110, :], ct[17][0:110, :], start=True,
               stop=True)
            emit_post(17)

    nc.compile()
    return nc


def _fold_conv_into_w1(conv_w: np.ndarray, w1: np.ndarray) -> np.ndarray:
    """W1f[784,100] such that x @ W1f == conv(x).reshape(B,676) @ w1."""
    c = np.zeros((NF, 26 * 26), dtype=np.float64)
    for di in range(3):
        for dj in range(3):
            ii, jj = np.meshgrid(np.arange(26), np.arange(26), indexing="ij")
            src = (ii + di) * 28 + (jj + dj)
            dst = ii * 26 + jj
            c[src.ravel(), dst.ravel()] += np.float64(conv_w[di, dj])
    return (c @ w1.astype(np.float64)).astype(np.float32)


def _prep_in_maps(x, conv_w, w1, b1, w2, b2, w3, b3):
    x = np.asarray(x, dtype=np.float32)
    conv_w = np.asarray(conv_w, dtype=np.float32)
    w1 = np.asarray(w1, dtype=np.float32)
    b1 = np.asarray(b1, dtype=np.float32)
    w2 = np.asarray(w2, dtype=np.float32)
    b2 = np.asarray(b2, dtype=np.float32)
    w3 = np.asarray(w3, dtype=np.float32)
    b3 = np.asarray(b3, dtype=np.float32)

    w1f = _fold_conv_into_w1(conv_w, w1)  # [784, 100]
    # mains shipped as e3m4 of 2*x (fewer subnormals), /2 folded into w1m
    # (exact in bf16); tail features shipped bf16 unscaled.
    w1m = (w1f[:128 * NKC] * 0.5).reshape(NKC, 128, H1)  # [6,128,100]
    w1t = w1f[128 * NKC:]  # [16, 100]

    blob = np.zeros((128, WBW), np.uint16)
    bv = lambda a: np.ascontiguousarray(a).astype(NP_BF16).view(np.uint16)
    # permuted output columns: [h1lo 0:96 | h2 96:106 | h1hi 106:110 | y]
    blob[:, C_CH0:C_CH0 + 96] = bv(w1m[0][:, 0:96])
    blob[:, C_CH0 + 106:C_CH0 + 110] = bv(w1m[0][:, 96:100])
    for k in range(1, NKC):
        c = C_CHK + 110 * (k - 1)
        blob[:, c:c + 96] = bv(w1m[k][:, 0:96])
        blob[:, c + 106:c + 110] = bv(w1m[k][:, 96:100])
    # combined stationary rows follow the moving-lane layout
    blob[0:96, C_CS + 96:C_CS + 106] = bv(w2[0:96])       # h1lo -> h2
    blob[96:106, C_CS + 110:C_CS + 120] = bv(w3)          # h2r -> y
    blob[106:110, C_CS + 96:C_CS + 106] = bv(w2[96:100])  # h1hi -> h2
    blob[110:126, C_CS:C_CS + 96] = bv(w1t[:, 0:96])      # xtail -> h1lo
    blob[110:126, C_CS + 106:C_CS + 110] = bv(w1t[:, 96:100])
    bias = np.concatenate([b1[0:96], b2, b1[96:100], b3]).astype(np.float32)
    blob[0:120, C_BIAS:C_BIAS + 2] = bias.reshape(120, 1).view(np.uint16)
    shared = {"wblob": blob.view(NP_BF16)}

    in_maps = []
    for core in range(N_CORES):
        xc = x[core * BC:(core + 1) * BC]  # [8192, 784] f32
        xct = xc.reshape(NT, TN, NF).transpose(0, 2, 1)  # [NT, NF, TN]
        xm = (np.ascontiguousarray(
            xct[:, :128 * NKC].reshape(NT, NKC, 128, TN).transpose(0, 2, 1, 3)
        ) * 2.0).astype(NP_E3)  # [NT, 128, NKC, TN]
        xt = np.ascontiguousarray(xct[:, 128 * NKC:]).astype(NP_BF16)
        in_maps.append({"xm": xm, "xt": xt, **shared})
    return in_maps


_NC = None


def _get_nc():
    global _NC
    if _NC is None:
        _NC = _build_nc()
    return _NC


def kernel(x, conv_w, w1, b1, w2, b2, w3, b3):
    in_maps = _prep_in_maps(x, conv_w, w1, b1, w2, b2, w3, b3)
    nc = _get_nc()
    res = run_bass_kernel_spmd(nc, in_maps, core_ids=list(range(N_CORES)))
    out = np.empty((B, HO), dtype=np.float32)
    for i in range(N_CORES):
        out[i * BC:(i + 1) * BC] = res.results[i]["yt"].T
    return out
